# revision 1
# baseline (speedup 1.0000x reference)
"""DropPart masking kernel for Trainium2 (8 NeuronCores, data-parallel over batch).

Problem: x (64, 256, 96, 32) f32. For each sample n and channel-group g (8 groups
x 32 channels), a keypoint defines a keep-box; if roll[n,g] < 0.5 the group's
channels are zeroed outside the box, else passed through unchanged.

Strategy:
  - Host computes the tiny per-(n,g) masks (96x32 each) from key_pts/roll in
    exact f32 arithmetic matching the reference, cast to bf16 (0/1 exact).
  - Batch dim sharded 8 samples/core. Per core the Bass/Tile kernel streams x
    through SBUF in [128ch, 3072hw] tiles; all 64 group-masks live in one
    [64, 3072] SBUF tile loaded once, and a per-(sample, half) one-hot matrix
    (TensorEngine matmul, K=64) expands them to per-channel masks in PSUM.
    The VectorEngine multiplies in place and the tile streams out. Loads are
    issued on the sync (SP) HWDGE ring, stores on the scalar (Activation)
    HWDGE ring so the two directions don't serialize on one descriptor ring.
  - Program is input-independent (mask values are data): one NEFF, SPMD on
    all 8 cores. Measured ~154 us/core = 331 GB/s/core (92% of the 358 GB/s
    HBM spec), equal to a loads+stores-only probe of the same traffic — i.e.
    at this hardware's memory floor; PE/DVE work is fully hidden.
"""

import numpy as np
import ml_dtypes

import concourse.bass as bass
import concourse.bacc as bacc
import concourse.tile as tile
from concourse import mybir
from concourse.bass_utils import run_bass_kernel_spmd

N, C, H, W = 64, 256, 96, 32
GROUPS = 8
P_DROP = 0.5
HW = H * W          # 3072
CHS = C // GROUPS   # 32
N_CORES = 8
NPC = N // N_CORES  # samples per core = 8
ROWS = NPC * C      # x rows per core = 2048
K = NPC * GROUPS    # mask rows per core = 64
NT = NPC * 2        # [128ch, HW] tiles per core = 16

_F32 = mybir.dt.float32
_BF16 = mybir.dt.bfloat16


def _build_module(reps: int = 1, loop_reps: int = 1):
    """loop_reps > 1 wraps the body in a For_i dynamic loop; -1 makes the
    trip count a runtime input "nreps" (both for benchmarking only; the
    back-edge adds ~2us per iteration)."""
    from contextlib import nullcontext

    nc = bacc.Bacc("TRN2", target_bir_lowering=False, debug=False)

    x_d = nc.dram_tensor("x", [ROWS, HW], _F32, kind="ExternalInput").ap()
    m_d = nc.dram_tensor("masks", [K, HW], _BF16, kind="ExternalInput").ap()
    e_d = nc.dram_tensor("eyes", [K, NT * 128], _BF16, kind="ExternalInput").ap()
    o_d = nc.dram_tensor("out", [ROWS, HW], _F32, kind="ExternalOutput").ap()
    r_d = None
    if loop_reps == -1:
        r_d = nc.dram_tensor("nreps", [1, 1], mybir.dt.int32, kind="ExternalInput").ap()

    PS = 1536  # psum chunk: 3 banks; 2 chunks per 128-channel tile

    with tile.TileContext(nc) as tc:
        with (
            tc.tile_pool(name="consts", bufs=1) as consts,
            tc.tile_pool(name="xpool", bufs=6) as xpool,
            tc.tile_pool(name="psum", bufs=2, space="PSUM") as psum,
        ):
            eyes = consts.tile([K, NT * 128], _BF16)
            nc.sync.dma_start(eyes[:], e_d[:])
            masks = consts.tile([K, HW], _BF16)
            nc.sync.dma_start(masks[:], m_d[:])

            if loop_reps == -1:
                rtile = consts.tile([1, 1], mybir.dt.int32)
                nc.sync.dma_start(rtile[:], r_d[:])
                loop_cm = tc.For_i(0, nc.values_load(rtile[0:1, 0:1]), 1)
            elif loop_reps > 1:
                loop_cm = tc.For_i(0, loop_reps, 1)
            else:
                loop_cm = nullcontext()
            with loop_cm:
                for _rep in range(reps):
                    for t in range(NT):  # (sample, channel-half) tiles
                        r0 = t * 128
                        xt = xpool.tile([128, HW], _F32)
                        nc.sync.dma_start(xt[:], x_d[r0 : r0 + 128, :])
                        for q in range(HW // PS):
                            pt = psum.tile([128, PS], _F32)
                            for j in range(PS // 512):
                                col = q * PS + j * 512
                                nc.tensor.matmul(
                                    pt[:, j * 512 : (j + 1) * 512],
                                    eyes[:, t * 128 : (t + 1) * 128],
                                    masks[:, col : col + 512],
                                    start=True,
                                    stop=True,
                                )
                            nc.vector.tensor_mul(
                                xt[:, q * PS : (q + 1) * PS],
                                xt[:, q * PS : (q + 1) * PS],
                                pt[:],
                            )
                        nc.scalar.dma_start(o_d[r0 : r0 + 128, :], xt[:])

    nc.compile()
    return nc


_NC = None


def _get_module():
    global _NC
    if _NC is None:
        _NC = _build_module()
    return _NC


def _host_masks(key_pts: np.ndarray, roll: np.ndarray) -> np.ndarray:
    """Per-(n,g) masks [N, GROUPS, H*W] in {0,1}, f32 math exactly as reference."""
    s = int(0.25 * W)
    kx = (key_pts[:, :GROUPS, 0] * np.float32(W)).astype(np.float32)
    ky = (key_pts[:, :GROUPS, 1] * np.float32(H)).astype(np.float32)
    cond = (roll[:, :GROUPS] < np.float32(P_DROP)) & (kx >= 0) & (ky >= 0)

    bx = np.floor(np.maximum(kx - s, np.float32(0.0)))
    ex = np.floor(np.minimum(kx + s, np.float32(W)))
    by = np.floor(np.maximum(ky - s, np.float32(0.0)))
    ey = np.floor(np.minimum(ky + s, np.float32(H)))

    xs = np.arange(W, dtype=np.float32)
    ys = np.arange(H, dtype=np.float32)
    inx = (xs[None, None, :] >= bx[:, :, None]) & (xs[None, None, :] < ex[:, :, None])
    iny = (ys[None, None, :] >= by[:, :, None]) & (ys[None, None, :] < ey[:, :, None])
    box = iny[:, :, :, None] & inx[:, :, None, :]  # [N, G, H, W] bool

    mask = np.where(cond[:, :, None, None], box, True)
    return mask.reshape(N, GROUPS, HW).astype(np.float32)


def _host_eyes() -> np.ndarray:
    """One-hot mask-row -> channel expanders, [K, NT*128] bf16.
    Column block t (= sample*2 + half) maps channel row m (0..127) to mask row
    sample*GROUPS + (half*128 + m)//CHS."""
    e = np.zeros((K, NT, 128), dtype=np.float32)
    for t in range(NT):
        s_idx, half = divmod(t, 2)
        for m in range(128):
            e[s_idx * GROUPS + (half * 128 + m) // CHS, t, m] = 1.0
    return e.reshape(K, NT * 128).astype(ml_dtypes.bfloat16)


def kernel(x: np.ndarray, key_pts: np.ndarray, roll: np.ndarray, **_kw) -> np.ndarray:
    x = np.ascontiguousarray(np.asarray(x, dtype=np.float32))
    key_pts = np.asarray(key_pts, dtype=np.float32)
    roll = np.asarray(roll, dtype=np.float32)

    masks = _host_masks(key_pts, roll).astype(ml_dtypes.bfloat16)
    eyes = _host_eyes()
    xr = x.reshape(N, C, HW)

    in_maps = []
    for c in range(N_CORES):
        sl = slice(c * NPC, (c + 1) * NPC)
        in_maps.append(
            {
                "x": np.ascontiguousarray(xr[sl]).reshape(ROWS, HW),
                "masks": np.ascontiguousarray(masks[sl]).reshape(K, HW),
                "eyes": eyes,
            }
        )

    nc = _get_module()
    res = run_bass_kernel_spmd(nc, in_maps, list(range(N_CORES))).results
    out = np.concatenate(
        [res[c]["out"].reshape(NPC, C, H, W) for c in range(N_CORES)], axis=0
    )
    return out

